# revision 12
# baseline (speedup 1.0000x reference)
"""Trainium2 Bass kernel for nn_DecoderTreeNN (Mem2Seq-style decoder with
tree-KB attention), data-parallel over batch across 8 NeuronCores.

Strategy (per core, 4 batch elements):
- story memory: one fused gather from a host-concatenated table
  C_cat[v] = [C_0[v]|C_1[v]|C_2[v]|C_3[v]] (4KB rows); the sum over the 4
  tokens per memory slot is done by the DMA itself via CCE-accumulate
  (indirect_dma_start compute_op=add).  m_story for all 4 hops lives in SBUF
  ([2048, 1024] fp32 per batch elem, double-buffered across batch elems).
- tree/GNN part: the 3-step masked scatter-add tree aggregation is linear in
  the node embeddings, so roots = sum_n r[t,n] * ne0[t,n] where the integer
  coefficients r derive from kb_fathers/kb_n_layers only (host index
  preprocessing).  Only nodes with r != 0 (~150/4160 per batch elem) have
  their value embeddings gathered; the weighted tree reduction is a single
  PE matmul against a host-built sparse weight matrix.
- hops: keys need E-major layout -> PE transposes of the SBUF-resident
  m_story planes; logits/softmax/value-reduction all stay on-chip.
- p_vocab: [4,512] @ W1^T streamed from HBM (host-pretransposed W1T).
"""
import os
import numpy as np

E = 256; V = 32000; NTYPE = 8; B = 32; M = 2048; MT = 4
NT = 64; NN = 64; L = 4; HOPS = 3; COMP_STEP = 3
NCORES = 8
BL = B // NCORES          # batch elems per core
MTILES = M // 128         # 16

_CACHE = {}


# ----------------------------------------------------------------------------
# host-side index preprocessing
# ----------------------------------------------------------------------------
def _r_coeffs(kb_fathers, kb_n_layers):
    """Integer path-count coefficients: roots[b,t] = sum_n r[b,t,n]*ne0[b,t,n].

    The reference does ne' = ne + diag(mask_s) A ne for s=0,1,2 with
    A[f,n] = [father(n)=f] and mask_s[p] = [2-n_layers[p] == s]; roots is row 0
    of the final ne.  r^T = e0^T (I+D2 A)(I+D1 A)(I+D0 A) evaluated with two
    gather steps along the father pointers.
    """
    Bb, Tt, Nn = kb_fathers.shape
    f = np.concatenate([kb_fathers, np.full((Bb, Tt, 1), Nn, kb_fathers.dtype)], 2)
    nl = np.concatenate([kb_n_layers, -np.ones((Bb, Tt, 1), kb_n_layers.dtype)], 2)
    nl_init = COMP_STEP - 1 - nl
    bI = np.arange(Bb)[:, None, None]
    tI = np.arange(Tt)[None, :, None]
    w = np.zeros((Bb, Tt, Nn + 1), np.float64)
    w[:, :, 0] = 1.0
    m2 = (nl_init[:, :, 0] == 2).astype(np.float64)
    w += m2[:, :, None] * (f == 0).astype(np.float64)
    for s in (1, 0):
        wm = w * (nl_init == s)
        w = w + wm[bI, tI, f % (Nn + 1)]
    w[:, :, Nn] = 0.0  # dump node embedding is identically zero
    return w


def _host_prep(inputs):
    story = np.ascontiguousarray(np.asarray(inputs['story'])).astype(np.int32)
    kbv = np.asarray(inputs['kb_values']).astype(np.int32)
    ktyp = np.asarray(inputs['kb_types']).astype(np.int32)
    kbf = np.asarray(inputs['kb_fathers']).astype(np.int64)
    kbl = np.asarray(inputs['kb_n_layers']).astype(np.int64)
    dec = np.asarray(inputs['decoder_input']).astype(np.int32)
    hs = np.asarray(inputs['hidden_states']).astype(np.float32)
    C = np.asarray(inputs['C_emb']).astype(np.float32)
    T = np.asarray(inputs['T_emb']).astype(np.float32)

    p = {}
    p['c_cat'] = np.ascontiguousarray(
        np.concatenate([C[0], C[1], C[2], C[3]], axis=1))          # [V, 1024]
    p['c0'] = np.ascontiguousarray(C[0])                            # [V, 256]
    p['t_tab'] = np.ascontiguousarray(T)                            # [8, 256]
    p['w1t'] = np.ascontiguousarray(np.asarray(inputs['W1_w']).astype(np.float32).T)
    p['w1b'] = np.asarray(inputs['W1_b']).astype(np.float32).reshape(1, V)
    p['wqt'] = np.ascontiguousarray(np.asarray(inputs['Wq']).astype(np.float32).T)
    p['wkt'] = np.ascontiguousarray(np.asarray(inputs['Wk']).astype(np.float32).T)
    p['wvt'] = np.ascontiguousarray(np.asarray(inputs['Wv']).astype(np.float32).T)
    p['wiht'] = np.ascontiguousarray(np.asarray(inputs['W_ih']).astype(np.float32).T)
    p['whht'] = np.ascontiguousarray(np.asarray(inputs['W_hh']).astype(np.float32).T)
    p['bih'] = np.asarray(inputs['b_ih']).astype(np.float32).reshape(1, 3 * E)
    p['bhh'] = np.asarray(inputs['b_hh']).astype(np.float32).reshape(1, 3 * E)

    # story offsets: [B, 128, 64] with col j = (mtile, t)
    p['story_off'] = np.ascontiguousarray(
        story.reshape(B, MTILES, 128, MT).transpose(0, 2, 1, 3).reshape(B, 128, MTILES * MT))

    # tree sparsification
    r = _r_coeffs(kbf, kbl)                                         # [B, NT, NN+1]
    nz = r != 0
    nnz = nz.sum(axis=(1, 2))
    nzcap = max(128, int(-(-int(nnz.max()) // 128) * 128))
    nzt = nzcap // 128
    kbv_off = np.zeros((B, 128, nzt * L), np.int32)
    ktype_off = np.zeros((B, 128, nzt), np.int32)
    wmat = np.zeros((B, 128, nzt * NT), np.float32)   # [p, tl*NT+t] = w[tl*128+p, t]
    for b in range(B):
        t_idx, n_idx = np.nonzero(r[b])
        k = len(t_idx)
        rows_v = kbv[b, t_idx, n_idx]                               # [k, 4]
        rows_t = ktyp[b, t_idx, n_idx]                              # [k]
        vv = np.zeros((nzcap, L), np.int32)
        vv[:k] = rows_v
        tt = np.zeros(nzcap, np.int32)
        tt[:k] = rows_t
        kbv_off[b] = vv.reshape(nzt, 128, L).transpose(1, 0, 2).reshape(128, nzt * L)
        ktype_off[b] = tt.reshape(nzt, 128).T
        wm = np.zeros((nzcap, NT), np.float32)
        wm[np.arange(k), t_idx] = r[b, t_idx, n_idx].astype(np.float32)
        wmat[b] = wm.reshape(nzt, 128, NT).transpose(1, 0, 2).reshape(128, nzt * NT)
    p['kbv_off'] = kbv_off
    p['ktype_off'] = ktype_off
    p['wmat'] = wmat
    p['dec_off'] = dec.reshape(B, 1)
    p['h_rows'] = np.ascontiguousarray(hs[0])                       # [B, 256]
    # hT_pack per core: [128, 2*BL] with col c*BL+b = h[b, c*128+p]
    hT = hs[0].reshape(B, 2, 128).transpose(2, 1, 0)                # [128, 2, B]
    p['hT'] = hT
    p['nzcap'] = nzcap
    return p


# ----------------------------------------------------------------------------
# device program
# ----------------------------------------------------------------------------
def _build(nzcap):
    import concourse.bass as bass
    import concourse.bacc as bacc
    import concourse.mybir as mybir
    import concourse.tile as tile
    from concourse.masks import make_identity

    fp32 = mybir.dt.float32
    i32 = mybir.dt.int32
    AF = mybir.ActivationFunctionType
    OP = mybir.AluOpType
    nzt = nzcap // 128

    nc = bacc.Bacc("TRN2", target_bir_lowering=False, debug=False, num_devices=NCORES)

    # --- DRAM tensors ---
    c_cat = nc.dram_tensor("c_cat", [V, 4 * E], fp32, kind="ExternalInput")
    c0 = nc.dram_tensor("c0", [V, E], fp32, kind="ExternalInput")
    t_tab = nc.dram_tensor("t_tab", [NTYPE, E], fp32, kind="ExternalInput")
    w1t = nc.dram_tensor("w1t", [2 * E, V], fp32, kind="ExternalInput")
    w1b = nc.dram_tensor("w1b", [1, V], fp32, kind="ExternalInput")
    wqt = nc.dram_tensor("wqt", [E, E], fp32, kind="ExternalInput")
    wkt = nc.dram_tensor("wkt", [E, E], fp32, kind="ExternalInput")
    wvt = nc.dram_tensor("wvt", [E, E], fp32, kind="ExternalInput")
    wiht = nc.dram_tensor("wiht", [E, 3 * E], fp32, kind="ExternalInput")
    whht = nc.dram_tensor("whht", [E, 3 * E], fp32, kind="ExternalInput")
    bih = nc.dram_tensor("bih", [1, 3 * E], fp32, kind="ExternalInput")
    bhh = nc.dram_tensor("bhh", [1, 3 * E], fp32, kind="ExternalInput")
    story_off = nc.dram_tensor("story_off", [BL, 128, MTILES * MT], i32, kind="ExternalInput")
    kbv_off = nc.dram_tensor("kbv_off", [BL, 128, nzt * L], i32, kind="ExternalInput")
    ktype_off = nc.dram_tensor("ktype_off", [BL, 128, nzt], i32, kind="ExternalInput")
    wmat = nc.dram_tensor("wmat", [BL, 128, nzt * NT], fp32, kind="ExternalInput")
    dec_off = nc.dram_tensor("dec_off", [BL, 1], i32, kind="ExternalInput")
    h_rows = nc.dram_tensor("h_rows", [BL, E], fp32, kind="ExternalInput")
    hT = nc.dram_tensor("hT", [128, 2 * BL], fp32, kind="ExternalInput")

    p_ptr_o = nc.dram_tensor("p_ptr_o", [BL, M], fp32, kind="ExternalOutput")
    p_vocab_o = nc.dram_tensor("p_vocab_o", [BL, V], fp32, kind="ExternalOutput")
    cur_o = nc.dram_tensor("cur_o", [BL, E], fp32, kind="ExternalOutput")

    with tile.TileContext(nc) as tc:
        with (
            tc.tile_pool(name="const", bufs=1) as cp,
            tc.tile_pool(name="work", bufs=2) as wp,
            tc.tile_pool(name="vstoryA", bufs=2) as vpa,
            tc.tile_pool(name="vstoryB", bufs=1) as vpb,
            tc.tile_pool(name="kt", bufs=1) as ktp,
            tc.tile_pool(name="w1rhs", bufs=4) as w1p,
        ):
            # ---- constants / weights resident in SBUF ----
            ident = cp.tile([128, 128], fp32)
            make_identity(nc, ident[:])
            ones14 = cp.tile([1, BL], fp32)
            nc.vector.memset(ones14[:], 1.0)
            ones128 = cp.tile([128, 1], fp32)
            nc.vector.memset(ones128[:], 1.0)

            def load_t(dram, shape, name):
                t = cp.tile(shape, fp32, tag=name)
                nc.sync.dma_start(t[:], dram[:])
                return t

            # E-chunked weight layouts: [128, 2*N] with block c = W[c*128:(c+1)*128, :]
            wqt_sb = cp.tile([128, 2 * E], fp32, tag="wqt")
            wkt_sb = cp.tile([128, 2 * E], fp32, tag="wkt")
            wvt_sb = cp.tile([128, 2 * E], fp32, tag="wvt")
            for c in range(2):
                nc.sync.dma_start(wqt_sb[:, c * E:(c + 1) * E], wqt[c * 128:(c + 1) * 128, :])
                nc.sync.dma_start(wkt_sb[:, c * E:(c + 1) * E], wkt[c * 128:(c + 1) * 128, :])
                nc.sync.dma_start(wvt_sb[:, c * E:(c + 1) * E], wvt[c * 128:(c + 1) * 128, :])
            wiht_sb = cp.tile([128, 2 * 3 * E], fp32, tag="wiht")
            whht_sb = cp.tile([128, 2 * 3 * E], fp32, tag="whht")
            for c in range(2):
                nc.sync.dma_start(wiht_sb[:, c * 3 * E:(c + 1) * 3 * E], wiht[c * 128:(c + 1) * 128, :])
                nc.sync.dma_start(whht_sb[:, c * 3 * E:(c + 1) * 3 * E], whht[c * 128:(c + 1) * 128, :])
            bih_sb = load_t(bih, [1, 3 * E], "bih")
            bhh_sb = load_t(bhh, [1, 3 * E], "bhh")
            h_sb = load_t(h_rows, [BL, E], "h_rows")
            hT_sb = load_t(hT, [128, 2 * BL], "hT")

            dec_sb = cp.tile([BL, 1], i32, tag="dec")
            nc.sync.dma_start(dec_sb[:], dec_off[:])

            # persistent across-batch tiles (all column-major, base partition 0)
            ucat = cp.tile([128, 4 * BL], fp32, tag="ucat")     # cols jc*BL+b
            af_col = cp.tile([128, 2 * BL], fp32, tag="afcol")  # cols ec*BL+b
            cur_col = cp.tile([128, 2 * BL], fp32, tag="curcol")

            # =========== GRU (batched over the BL local batch elems) ===========
            with tc.tile_pool(name="psA", bufs=1, space="PSUM") as psA:
                x_rows = wp.tile([BL, E], fp32, tag="xrows")
                nc.gpsimd.indirect_dma_start(
                    out=x_rows[:], out_offset=None, in_=c0[:],
                    in_offset=bass.IndirectOffsetOnAxis(ap=dec_sb[:, 0:1], axis=0))
                xcol = wp.tile([128, 2 * BL], fp32, tag="xcol")
                for c in range(2):
                    tp = psA.tile([128, BL], fp32, tag="tpA")
                    nc.tensor.transpose(tp[:], x_rows[:, c * 128:(c + 1) * 128], ident[0:BL, 0:BL])
                    nc.vector.tensor_copy(xcol[:, c * BL:(c + 1) * BL], tp[:])

                gi_a = psA.tile([BL, 512], fp32, tag="gia")
                gi_b = psA.tile([BL, 256], fp32, tag="gib")
                gh_a = psA.tile([BL, 512], fp32, tag="gha")
                gh_b = psA.tile([BL, 256], fp32, tag="ghb")
                for c in range(2):
                    st, sp = (c == 0), (c == 1)
                    nc.tensor.matmul(gi_a[:], lhsT=xcol[:, c * BL:(c + 1) * BL],
                                     rhs=wiht_sb[:, c * 768:c * 768 + 512], start=st, stop=False)
                    nc.tensor.matmul(gi_b[:], lhsT=xcol[:, c * BL:(c + 1) * BL],
                                     rhs=wiht_sb[:, c * 768 + 512:c * 768 + 768], start=st, stop=False)
                    nc.tensor.matmul(gh_a[:], lhsT=hT_sb[:, c * BL:(c + 1) * BL],
                                     rhs=whht_sb[:, c * 768:c * 768 + 512], start=st, stop=False)
                    nc.tensor.matmul(gh_b[:], lhsT=hT_sb[:, c * BL:(c + 1) * BL],
                                     rhs=whht_sb[:, c * 768 + 512:c * 768 + 768], start=st, stop=False)
                # biases via K=1 matmul with ones
                nc.tensor.matmul(gi_a[:], lhsT=ones14[:], rhs=bih_sb[:, 0:512], start=False, stop=True)
                nc.tensor.matmul(gi_b[:], lhsT=ones14[:], rhs=bih_sb[:, 512:768], start=False, stop=True)
                nc.tensor.matmul(gh_a[:], lhsT=ones14[:], rhs=bhh_sb[:, 0:512], start=False, stop=True)
                nc.tensor.matmul(gh_b[:], lhsT=ones14[:], rhs=bhh_sb[:, 512:768], start=False, stop=True)

                r_sb = wp.tile([BL, E], fp32, tag="r")
                z_sb = wp.tile([BL, E], fp32, tag="z")
                n_sb = wp.tile([BL, E], fp32, tag="n")
                tmp = wp.tile([BL, E], fp32, tag="gtmp")
                gh_sb = wp.tile([BL, 3 * E], fp32, tag="ghsb")
                h_new = cp.tile([BL, E], fp32, tag="hnew")
                nc.vector.tensor_copy(gh_sb[:, 0:512], gh_a[:])
                nc.vector.tensor_copy(gh_sb[:, 512:768], gh_b[:])
                nc.vector.tensor_add(tmp[:], gi_a[:, 0:E], gh_sb[:, 0:E])
                nc.scalar.activation(r_sb[:], tmp[:], AF.Sigmoid)
                nc.vector.tensor_add(tmp[:], gi_a[:, E:2 * E], gh_sb[:, E:2 * E])
                nc.scalar.activation(z_sb[:], tmp[:], AF.Sigmoid)
                nc.vector.tensor_mul(tmp[:], r_sb[:], gh_sb[:, 2 * E:3 * E])
                nc.vector.tensor_add(tmp[:], tmp[:], gi_b[:])
                nc.scalar.activation(n_sb[:], tmp[:], AF.Tanh)
                # h_new = n + z*(h - n)
                nc.vector.tensor_sub(tmp[:], h_sb[:], n_sb[:])
                nc.vector.tensor_mul(tmp[:], tmp[:], z_sb[:])
                nc.vector.tensor_add(h_new[:], n_sb[:], tmp[:])

            # =========== per-batch: tree attention + story hops ===========
            with tc.tile_pool(name="psB", bufs=2, space="PSUM") as psB:

                def transpose_to(dst_ap, src_ap, psq=128):
                    pp = psB.tile([src_ap.shape[-1], src_ap.shape[0]], fp32, tag="tp")
                    nc.tensor.transpose(pp[:], src_ap,
                                        ident[0:src_ap.shape[0], 0:src_ap.shape[0]])
                    nc.vector.tensor_copy(dst_ap, pp[:])

                for b in range(BL):
                    # ---- tree/KB attention ----
                    kb_sb = wp.tile([128, nzt * L], i32, tag="kbo")
                    nc.sync.dma_start(kb_sb[:], kbv_off[b])
                    kt_sb = wp.tile([128, nzt], i32, tag="kto")
                    nc.sync.dma_start(kt_sb[:], ktype_off[b])
                    wm_sb = wp.tile([128, nzt * NT], fp32, tag="wm")
                    nc.sync.dma_start(wm_sb[:], wmat[b])

                    gkb = []
                    for tl in range(nzt):
                        g = wp.tile([128, E], fp32, tag=f"gkb{tl}")
                        for l in range(L):
                            nc.gpsimd.indirect_dma_start(
                                out=g[:], out_offset=None, in_=c0[:],
                                in_offset=bass.IndirectOffsetOnAxis(
                                    ap=kb_sb[:, tl * L + l:tl * L + l + 1], axis=0),
                                compute_op=(OP.bypass if l == 0 else OP.add))
                        trow = wp.tile([128, E], fp32, tag=f"trow{tl}")
                        nc.gpsimd.indirect_dma_start(
                            out=trow[:], out_offset=None, in_=t_tab[:],
                            in_offset=bass.IndirectOffsetOnAxis(
                                ap=kt_sb[:, tl:tl + 1], axis=0))
                        nc.vector.tensor_mul(g[:], g[:], trow[:])
                        gkb.append(g)

                    # rootsT [e, t] in 2 e-chunks
                    rootsT = wp.tile([128, 2 * NT], fp32, tag="rootsT")
                    for ec in range(2):
                        rp = psB.tile([128, NT], fp32, tag="attn")
                        for tl in range(nzt):
                            nc.tensor.matmul(rp[:], lhsT=gkb[tl][:, ec * 128:(ec + 1) * 128],
                                             rhs=wm_sb[:, tl * NT:(tl + 1) * NT],
                                             start=(tl == 0), stop=(tl == nzt - 1))
                        nc.vector.tensor_copy(rootsT[:, ec * NT:(ec + 1) * NT], rp[:])

                    # attention bias: -1e9 where sum_e roots[t,:] == 0
                    bias_ps = psB.tile([NT, 1], fp32, tag="attn")
                    for ec in range(2):
                        nc.tensor.matmul(bias_ps[:], lhsT=rootsT[:, ec * NT:(ec + 1) * NT],
                                         rhs=ones128[:], start=(ec == 0), stop=(ec == 1))
                    biasneg = wp.tile([NT, 1], fp32, tag="biasneg")
                    nc.vector.tensor_scalar(out=biasneg[:], in0=bias_ps[:],
                                            scalar1=0.0, scalar2=-1e9,
                                            op0=OP.is_equal, op1=OP.mult)

                    # keyrT [e', t] = Wk @ rootsT ; valr [t, e'] = rootsT^T @ WvT
                    keyrT = wp.tile([128, 2 * NT], fp32, tag="keyrT")
                    for ecp in range(2):
                        kp = psB.tile([128, NT], fp32, tag="attn")
                        for ec in range(2):
                            nc.tensor.matmul(
                                kp[:], lhsT=wkt_sb[:, ec * E + ecp * 128:ec * E + (ecp + 1) * 128],
                                rhs=rootsT[:, ec * NT:(ec + 1) * NT],
                                start=(ec == 0), stop=(ec == 1))
                        nc.vector.tensor_copy(keyrT[:, ecp * NT:(ecp + 1) * NT], kp[:])
                    valr = wp.tile([NT, E], fp32, tag="valr")
                    vp_ps = psB.tile([NT, E], fp32, tag="attn")
                    for ec in range(2):
                        nc.tensor.matmul(vp_ps[:], lhsT=rootsT[:, ec * NT:(ec + 1) * NT],
                                         rhs=wvt_sb[:, ec * E:(ec + 1) * E],
                                         start=(ec == 0), stop=(ec == 1))
                    nc.vector.tensor_copy(valr[:], vp_ps[:])

                    # query (column layout) for this batch elem
                    qcol = wp.tile([128, 2], fp32, tag="qcol")
                    for ecp in range(2):
                        qp = psB.tile([128, 1], fp32, tag="attn")
                        for ec in range(2):
                            nc.tensor.matmul(
                                qp[:], lhsT=wqt_sb[:, ec * E + ecp * 128:ec * E + (ecp + 1) * 128],
                                rhs=hT_sb[:, ec * BL + b:ec * BL + b + 1],
                                start=(ec == 0), stop=(ec == 1))
                        nc.vector.tensor_copy(qcol[:, ecp:ecp + 1], qp[:])

                    lg_ps = psB.tile([NT, 1], fp32, tag="attn")
                    for ecp in range(2):
                        nc.tensor.matmul(lg_ps[:], lhsT=keyrT[:, ecp * NT:(ecp + 1) * NT],
                                         rhs=qcol[:, ecp:ecp + 1],
                                         start=(ecp == 0), stop=(ecp == 1))
                    exps = wp.tile([NT, 1], fp32, tag="exps")
                    nc.scalar.activation(exps[:], lg_ps[:], AF.Exp, bias=biasneg[:])
                    z_ps = psB.tile([1, 1], fp32, tag="attn")
                    nc.tensor.matmul(z_ps[:], lhsT=exps[:], rhs=ones128[0:NT, :],
                                     start=True, stop=True)
                    zinv = wp.tile([1, 1], fp32, tag="zinv")
                    nc.vector.reciprocal(zinv[:], z_ps[:])
                    af_ps = psB.tile([1, E], fp32, tag="attn")
                    nc.tensor.matmul(af_ps[:], lhsT=exps[:], rhs=valr[:], start=True, stop=True)
                    af_row = wp.tile([1, E], fp32, tag="afrow")
                    nc.vector.tensor_scalar(out=af_row[:], in0=af_ps[:],
                                            scalar1=zinv[:], scalar2=None, op0=OP.mult)
                    for ec in range(2):
                        transpose_to(af_col[:, ec * BL + b:ec * BL + b + 1],
                                     af_row[:, ec * 128:(ec + 1) * 128])

                # cur_state (column layout) = h_new + attn_feat
                hnew_col = wp.tile([128, 2 * BL], fp32, tag="hnewcol")
                for ec in range(2):
                    transpose_to(hnew_col[:, ec * BL:(ec + 1) * BL],
                                 h_new[:, ec * 128:(ec + 1) * 128])
                nc.vector.tensor_add(cur_col[:], hnew_col[:], af_col[:])
                # back to rows for the cur_state output
                cur_rows = wp.tile([BL, E], fp32, tag="currows")
                for ec in range(2):
                    transpose_to(cur_rows[:, ec * 128:(ec + 1) * 128],
                                 cur_col[:, ec * BL:(ec + 1) * BL])
                nc.sync.dma_start(cur_o[:], cur_rows[:])

                # ---- story hops per batch elem ----
                for b in range(BL):
                    so_sb = wp.tile([128, MTILES * MT], i32, tag="so")
                    nc.sync.dma_start(so_sb[:], story_off[b])
                    vs = []
                    for mt in range(MTILES):
                        vtile = (vpa if mt < 8 else vpb).tile([128, 4 * E], fp32, tag=f"v{mt}")
                        for t in range(MT):
                            nc.gpsimd.indirect_dma_start(
                                out=vtile[:], out_offset=None, in_=c_cat[:],
                                in_offset=bass.IndirectOffsetOnAxis(
                                    ap=so_sb[:, mt * MT + t:mt * MT + t + 1], axis=0),
                                compute_op=(OP.bypass if t == 0 else OP.add))
                        vs.append(vtile)

                    # u in column layout [128, 2] (cols = e-chunks), base partition 0
                    ucol = wp.tile([128, 2], fp32, tag="ucol")
                    for ec in range(2):
                        nc.vector.tensor_copy(ucol[:, ec:ec + 1],
                                              cur_col[:, ec * BL + b:ec * BL + b + 1])
                        nc.vector.tensor_copy(ucat[:, ec * BL + b:ec * BL + b + 1],
                                              cur_col[:, ec * BL + b:ec * BL + b + 1])

                    for hop in range(HOPS):
                        # keys: transpose m_story[hop] -> [e, m] and dot with u
                        kt_t = ktp.tile([128, 2 * M], fp32, tag="kt")
                        for mt in range(MTILES):
                            for ec in range(2):
                                transpose_to(
                                    kt_t[:, ec * M + mt * 128:ec * M + (mt + 1) * 128],
                                    vs[mt][:, hop * E + ec * 128:hop * E + (ec + 1) * 128])
                        s_ps = psB.tile([128, MTILES], fp32, tag="s")
                        for mt in range(MTILES):
                            for ec in range(2):
                                nc.tensor.matmul(
                                    s_ps[:, mt:mt + 1],
                                    lhsT=kt_t[:, ec * M + mt * 128:ec * M + (mt + 1) * 128],
                                    rhs=ucol[:, ec:ec + 1],
                                    start=(ec == 0), stop=(ec == 1))

                        if hop == HOPS - 1:
                            # p_ptr = raw logits of the last hop
                            s_sb = wp.tile([128, MTILES], fp32, tag="ssb")
                            nc.vector.tensor_copy(s_sb[:], s_ps[:])
                            pp_ps = psB.tile([MTILES, 128], fp32, tag="tp")
                            nc.tensor.transpose(pp_ps[:], s_sb[:], ident[:, :])
                            pptr_sb = wp.tile([MTILES, 128], fp32, tag="pptr")
                            nc.vector.tensor_copy(pptr_sb[:], pp_ps[:])
                            nc.sync.dma_start(
                                p_ptr_o[:].rearrange("b (s c) -> b s c", s=MTILES)[b],
                                pptr_sb[:])
                            break

                        exp_sb = wp.tile([128, MTILES], fp32, tag="expsb")
                        rowsum = wp.tile([128, 1], fp32, tag="rowsum")
                        nc.scalar.activation(exp_sb[:], s_ps[:], AF.Exp, accum_out=rowsum[:])
                        z2_ps = psB.tile([1, 1], fp32, tag="s")
                        nc.tensor.matmul(z2_ps[:], lhsT=rowsum[:], rhs=ones128[:],
                                         start=True, stop=True)
                        zinv2 = wp.tile([1, 1], fp32, tag="zinv2")
                        nc.vector.reciprocal(zinv2[:], z2_ps[:])

                        ok_ps = psB.tile([1, E], fp32, tag="ok")
                        for mt in range(MTILES):
                            nc.tensor.matmul(ok_ps[:], lhsT=exp_sb[:, mt:mt + 1],
                                             rhs=vs[mt][:, (hop + 1) * E:(hop + 2) * E],
                                             start=(mt == 0), stop=(mt == MTILES - 1))
                        okn = wp.tile([1, E], fp32, tag="okn")
                        nc.vector.tensor_scalar(out=okn[:], in0=ok_ps[:],
                                                scalar1=zinv2[:], scalar2=None, op0=OP.mult)
                        okn_col = wp.tile([128, 2], fp32, tag="okncol")
                        for ec in range(2):
                            transpose_to(okn_col[:, ec:ec + 1], okn[:, ec * 128:(ec + 1) * 128])
                        if hop == 0:
                            nc.vector.tensor_copy(ucat[:, 2 * BL + b:2 * BL + b + 1],
                                                  okn_col[:, 0:1])
                            nc.vector.tensor_copy(ucat[:, 3 * BL + b:3 * BL + b + 1],
                                                  okn_col[:, 1:2])
                        # u <- u + o_k  (column space)
                        ucol_next = wp.tile([128, 2], fp32, tag="ucol")
                        nc.vector.tensor_add(ucol_next[:], ucol[:], okn_col[:])
                        ucol = ucol_next

            # =========== p_vocab = [u0|o_k0] @ W1^T + b ===========
            with tc.tile_pool(name="psC", bufs=2, space="PSUM") as psC:
                NCH = (V + 511) // 512
                for nch in range(NCH):
                    n0 = nch * 512
                    nsz = min(512, V - n0)
                    pv = psC.tile([BL, nsz], fp32, tag="pv")
                    for jc in range(4):
                        wtile = w1p.tile([128, nsz], fp32, tag="w1")
                        nc.sync.dma_start(wtile[:], w1t[jc * 128:(jc + 1) * 128, n0:n0 + nsz])
                        nc.tensor.matmul(pv[:], lhsT=ucat[:, jc * BL:(jc + 1) * BL],
                                         rhs=wtile[:], start=(jc == 0), stop=False)
                    btile = w1p.tile([1, nsz], fp32, tag="w1bt")
                    nc.sync.dma_start(btile[:], w1b[:, n0:n0 + nsz])
                    nc.tensor.matmul(pv[:], lhsT=ones14[:], rhs=btile[:],
                                     start=False, stop=True)
                    pvs = wp.tile([BL, nsz], fp32, tag="pvs")
                    nc.vector.tensor_copy(pvs[:], pv[:])
                    nc.sync.dma_start(p_vocab_o[:, n0:n0 + nsz], pvs[:])

    nc.compile()
    return nc


# ----------------------------------------------------------------------------
# entry point
# ----------------------------------------------------------------------------
def kernel(**inputs):
    from concourse import bass_utils

    p = _host_prep(inputs)
    nzcap = p['nzcap']
    if nzcap not in _CACHE:
        _CACHE[nzcap] = _build(nzcap)
    nc = _CACHE[nzcap]

    shared = {k: p[k] for k in ('c_cat', 'c0', 't_tab', 'w1t', 'w1b', 'wqt', 'wkt',
                                'wvt', 'wiht', 'whht', 'bih', 'bhh')}
    in_maps = []
    for c in range(NCORES):
        sl = slice(c * BL, (c + 1) * BL)
        m = dict(shared)
        m['story_off'] = p['story_off'][sl]
        m['kbv_off'] = p['kbv_off'][sl]
        m['ktype_off'] = p['ktype_off'][sl]
        m['wmat'] = p['wmat'][sl]
        m['dec_off'] = p['dec_off'][sl]
        m['h_rows'] = p['h_rows'][sl]
        m['hT'] = np.ascontiguousarray(p['hT'][:, :, sl].reshape(128, 2 * BL))
        in_maps.append(m)

    res = bass_utils.run_bass_kernel_spmd(nc, in_maps, core_ids=list(range(NCORES)))
    if res.exec_time_ns is not None:
        kernel.last_exec_time_ns = res.exec_time_ns

    p_ptr = np.concatenate([res.results[c]["p_ptr_o"] for c in range(NCORES)], 0)
    p_vocab = np.concatenate([res.results[c]["p_vocab_o"] for c in range(NCORES)], 0)
    cur = np.concatenate([res.results[c]["cur_o"] for c in range(NCORES)], 0)
    return p_ptr, p_vocab, cur[None]


kernel.last_exec_time_ns = None


# revision 36
# speedup vs baseline: 1.5847x; 1.5847x over previous
"""Trainium2 Bass kernel for nn_DecoderTreeNN (Mem2Seq-style decoder with
tree-KB attention), data-parallel over batch across 8 NeuronCores.

Strategy (per core, 4 batch elements):
- story memory: one fused gather from a host-concatenated table
  C_cat[v] = [C_0[v]|C_1[v]|C_2[v]|C_3[v]] (4KB rows); the sum over the 4
  tokens per memory slot is done by the DMA itself via CCE-accumulate
  (indirect_dma_start compute_op=add).  m_story for all 4 hops lives in SBUF
  ([2048, 1024] fp32 per batch elem, double-buffered across batch elems).
- tree/GNN part: the 3-step masked scatter-add tree aggregation is linear in
  the node embeddings, so roots = sum_n r[t,n] * ne0[t,n] where the integer
  coefficients r derive from kb_fathers/kb_n_layers only (host index
  preprocessing).  Only nodes with r != 0 (~150/4160 per batch elem) have
  their value embeddings gathered; the weighted tree reduction is a single
  PE matmul against a host-built sparse weight matrix.
- hops: keys need E-major layout -> PE transposes of the SBUF-resident
  m_story planes; logits/softmax/value-reduction all stay on-chip.
- p_vocab: [4,512] @ W1^T streamed from HBM (host-pretransposed W1T).
"""
import os
import numpy as np

E = 256; V = 32000; NTYPE = 8; B = 32; M = 2048; MT = 4
NT = 64; NN = 64; L = 4; HOPS = 3; COMP_STEP = 3
NCORES = 8
BL = B // NCORES          # batch elems per core
MTILES = M // 128         # 16

_CACHE = {}


# ----------------------------------------------------------------------------
# host-side index preprocessing
# ----------------------------------------------------------------------------
def _r_coeffs(kb_fathers, kb_n_layers):
    """Integer path-count coefficients: roots[b,t] = sum_n r[b,t,n]*ne0[b,t,n].

    The reference does ne' = ne + diag(mask_s) A ne for s=0,1,2 with
    A[f,n] = [father(n)=f] and mask_s[p] = [2-n_layers[p] == s]; roots is row 0
    of the final ne.  r^T = e0^T (I+D2 A)(I+D1 A)(I+D0 A) evaluated with two
    gather steps along the father pointers.
    """
    Bb, Tt, Nn = kb_fathers.shape
    f = np.concatenate([kb_fathers, np.full((Bb, Tt, 1), Nn, kb_fathers.dtype)], 2)
    nl = np.concatenate([kb_n_layers, -np.ones((Bb, Tt, 1), kb_n_layers.dtype)], 2)
    nl_init = COMP_STEP - 1 - nl
    bI = np.arange(Bb)[:, None, None]
    tI = np.arange(Tt)[None, :, None]
    w = np.zeros((Bb, Tt, Nn + 1), np.float64)
    w[:, :, 0] = 1.0
    m2 = (nl_init[:, :, 0] == 2).astype(np.float64)
    w += m2[:, :, None] * (f == 0).astype(np.float64)
    for s in (1, 0):
        wm = w * (nl_init == s)
        w = w + wm[bI, tI, f % (Nn + 1)]
    w[:, :, Nn] = 0.0  # dump node embedding is identically zero
    return w


def _host_prep(inputs):
    story = np.ascontiguousarray(np.asarray(inputs['story'])).astype(np.int32)
    kbv = np.asarray(inputs['kb_values']).astype(np.int32)
    ktyp = np.asarray(inputs['kb_types']).astype(np.int32)
    kbf = np.asarray(inputs['kb_fathers']).astype(np.int64)
    kbl = np.asarray(inputs['kb_n_layers']).astype(np.int64)
    dec = np.asarray(inputs['decoder_input']).astype(np.int32)
    hs = np.asarray(inputs['hidden_states']).astype(np.float32)
    C = np.asarray(inputs['C_emb']).astype(np.float32)
    T = np.asarray(inputs['T_emb']).astype(np.float32)

    p = {}
    p['c_cat'] = np.ascontiguousarray(
        np.concatenate([C[0], C[1], C[2], C[3]], axis=1))          # [V, 1024]
    p['c0'] = np.ascontiguousarray(C[0])                            # [V, 256]
    p['t_tab'] = np.ascontiguousarray(T)                            # [8, 256]
    p['w1t'] = np.ascontiguousarray(np.asarray(inputs['W1_w']).astype(np.float32).T)
    p['w1b'] = np.asarray(inputs['W1_b']).astype(np.float32).reshape(1, V)
    p['wqt'] = np.ascontiguousarray(np.asarray(inputs['Wq']).astype(np.float32).T)
    p['wkt'] = np.ascontiguousarray(np.asarray(inputs['Wk']).astype(np.float32).T)
    p['wvt'] = np.ascontiguousarray(np.asarray(inputs['Wv']).astype(np.float32).T)
    p['wiht'] = np.ascontiguousarray(np.asarray(inputs['W_ih']).astype(np.float32).T)
    p['whht'] = np.ascontiguousarray(np.asarray(inputs['W_hh']).astype(np.float32).T)
    p['bih'] = np.asarray(inputs['b_ih']).astype(np.float32).reshape(1, 3 * E)
    p['bhh'] = np.asarray(inputs['b_hh']).astype(np.float32).reshape(1, 3 * E)

    # story offsets: [B, 128, 64] with col j = (mtile, t)
    p['story_off'] = np.ascontiguousarray(
        story.reshape(B, MTILES, 128, MT).transpose(0, 2, 1, 3).reshape(B, 128, MTILES * MT))

    # tree sparsification
    r = _r_coeffs(kbf, kbl)                                         # [B, NT, NN+1]
    nz = r != 0
    nnz = nz.sum(axis=(1, 2))
    nzcap = max(128, int(-(-int(nnz.max()) // 128) * 128))
    nzt = nzcap // 128
    kbv_off = np.zeros((B, 128, nzt * L), np.int32)
    ktype_off = np.zeros((B, 128, nzt), np.int32)
    wmat = np.zeros((B, 128, nzt * NT), np.float32)   # [p, tl*NT+t] = w[tl*128+p, t]
    for b in range(B):
        t_idx, n_idx = np.nonzero(r[b])
        k = len(t_idx)
        rows_v = kbv[b, t_idx, n_idx]                               # [k, 4]
        rows_t = ktyp[b, t_idx, n_idx]                              # [k]
        vv = np.zeros((nzcap, L), np.int32)
        vv[:k] = rows_v
        tt = np.zeros(nzcap, np.int32)
        tt[:k] = rows_t
        kbv_off[b] = vv.reshape(nzt, 128, L).transpose(1, 0, 2).reshape(128, nzt * L)
        ktype_off[b] = tt.reshape(nzt, 128).T
        wm = np.zeros((nzcap, NT), np.float32)
        wm[np.arange(k), t_idx] = r[b, t_idx, n_idx].astype(np.float32)
        wmat[b] = wm.reshape(nzt, 128, NT).transpose(1, 0, 2).reshape(128, nzt * NT)
    p['kbv_off'] = kbv_off
    p['ktype_off'] = ktype_off
    p['wmat'] = wmat
    p['dec_off'] = dec.reshape(B, 1)
    p['h_rows'] = np.ascontiguousarray(hs[0])                       # [B, 256]
    # hT_pack per core: [128, 2*BL] with col c*BL+b = h[b, c*128+p]
    hT = hs[0].reshape(B, 2, 128).transpose(2, 1, 0)                # [128, 2, B]
    p['hT'] = hT
    p['nzcap'] = nzcap
    return p


# ----------------------------------------------------------------------------
# device program
# ----------------------------------------------------------------------------
def _build(nzcap):
    import concourse.bass as bass
    import concourse.bacc as bacc
    import concourse.mybir as mybir
    import concourse.tile as tile
    from concourse.masks import make_identity

    fp32 = mybir.dt.float32
    i32 = mybir.dt.int32
    AF = mybir.ActivationFunctionType
    OP = mybir.AluOpType
    nzt = nzcap // 128

    nc = bacc.Bacc("TRN2", target_bir_lowering=False, debug=False, num_devices=NCORES)

    # --- DRAM tensors ---
    c_cat = nc.dram_tensor("c_cat", [V, 4 * E], fp32, kind="ExternalInput")
    c0 = nc.dram_tensor("c0", [V, E], fp32, kind="ExternalInput")
    t_tab = nc.dram_tensor("t_tab", [NTYPE, E], fp32, kind="ExternalInput")
    VS = V // NCORES   # vocab shard per core for the W1 projection
    w1t = nc.dram_tensor("w1t", [2 * E, VS], fp32, kind="ExternalInput")
    w1b = nc.dram_tensor("w1b", [1, VS], fp32, kind="ExternalInput")
    wqt = nc.dram_tensor("wqt", [E, E], fp32, kind="ExternalInput")
    wkt = nc.dram_tensor("wkt", [E, E], fp32, kind="ExternalInput")
    wvt = nc.dram_tensor("wvt", [E, E], fp32, kind="ExternalInput")
    wiht = nc.dram_tensor("wiht", [E, 3 * E], fp32, kind="ExternalInput")
    whht = nc.dram_tensor("whht", [E, 3 * E], fp32, kind="ExternalInput")
    bih = nc.dram_tensor("bih", [1, 3 * E], fp32, kind="ExternalInput")
    bhh = nc.dram_tensor("bhh", [1, 3 * E], fp32, kind="ExternalInput")
    story_off = nc.dram_tensor("story_off", [BL, 128, MTILES * MT], i32, kind="ExternalInput")
    kbv_off = nc.dram_tensor("kbv_off", [BL, 128, nzt * L], i32, kind="ExternalInput")
    ktype_off = nc.dram_tensor("ktype_off", [BL, 128, nzt], i32, kind="ExternalInput")
    wmat = nc.dram_tensor("wmat", [BL, 128, nzt * NT], fp32, kind="ExternalInput")
    dec_off = nc.dram_tensor("dec_off", [BL, 1], i32, kind="ExternalInput")
    h_rows = nc.dram_tensor("h_rows", [BL, E], fp32, kind="ExternalInput")
    hT = nc.dram_tensor("hT", [128, 2 * BL], fp32, kind="ExternalInput")

    p_ptr_o = nc.dram_tensor("p_ptr_o", [BL, M], fp32, kind="ExternalOutput")
    p_vocab_o = nc.dram_tensor("p_vocab_o", [B, VS], fp32, kind="ExternalOutput")
    cur_o = nc.dram_tensor("cur_o", [BL, E], fp32, kind="ExternalOutput")

    with tile.TileContext(nc) as tc:
        with (
            tc.tile_pool(name="const", bufs=1) as cp,
            tc.tile_pool(name="work", bufs=2) as wp,
            tc.tile_pool(name="vstory", bufs=18) as vp,
            tc.tile_pool(name="graw", bufs=2) as gp,
            tc.tile_pool(name="w1rhs", bufs=4) as w1p,
        ):
            # ---- constants / weights resident in SBUF ----
            ident = cp.tile([128, 128], fp32)
            make_identity(nc, ident[:])
            ones14 = cp.tile([1, BL], fp32)
            nc.vector.memset(ones14[:], 1.0)
            ones132 = cp.tile([1, B], fp32)
            nc.vector.memset(ones132[:], 1.0)
            ones128 = cp.tile([128, 1], fp32)
            nc.vector.memset(ones128[:], 1.0)
            ones_row = cp.tile([1, 128], fp32)
            nc.vector.memset(ones_row[:], 1.0)

            def load_t(dram, shape, name):
                t = cp.tile(shape, fp32, tag=name)
                nc.sync.dma_start(t[:], dram[:])
                return t

            # E-chunked weight layouts: [128, 2*N] with block c = W[c*128:(c+1)*128, :]
            wqt_sb = cp.tile([128, 2 * E], fp32, tag="wqt")
            wkt_sb = cp.tile([128, 2 * E], fp32, tag="wkt")
            wvt_sb = cp.tile([128, 2 * E], fp32, tag="wvt")
            for c in range(2):
                nc.sync.dma_start(wqt_sb[:, c * E:(c + 1) * E], wqt[c * 128:(c + 1) * 128, :])
                nc.sync.dma_start(wkt_sb[:, c * E:(c + 1) * E], wkt[c * 128:(c + 1) * 128, :])
                nc.sync.dma_start(wvt_sb[:, c * E:(c + 1) * E], wvt[c * 128:(c + 1) * 128, :])
            wiht_sb = cp.tile([128, 2 * 3 * E], fp32, tag="wiht")
            whht_sb = cp.tile([128, 2 * 3 * E], fp32, tag="whht")
            for c in range(2):
                nc.sync.dma_start(wiht_sb[:, c * 3 * E:(c + 1) * 3 * E], wiht[c * 128:(c + 1) * 128, :])
                nc.sync.dma_start(whht_sb[:, c * 3 * E:(c + 1) * 3 * E], whht[c * 128:(c + 1) * 128, :])
            bih_sb = load_t(bih, [1, 3 * E], "bih")
            bhh_sb = load_t(bhh, [1, 3 * E], "bhh")
            h_sb = load_t(h_rows, [BL, E], "h_rows")
            hT_sb = load_t(hT, [128, 2 * BL], "hT")

            dec_sb = cp.tile([BL, 1], i32, tag="dec")
            nc.sync.dma_start(dec_sb[:], dec_off[:])

            # persistent across-batch tiles (all column-major, base partition 0)
            ucat = cp.tile([128, 4 * BL], fp32, tag="ucat")     # cols jc*BL+b
            af_col = cp.tile([128, 2 * BL], fp32, tag="afcol")  # cols ec*BL+b
            cur_col = cp.tile([128, 2 * BL], fp32, tag="curcol")

            # =========== GRU (batched over the BL local batch elems) ===========
            with tc.tile_pool(name="psA", bufs=1, space="PSUM") as psA:
                x_rows = wp.tile([BL, E], fp32, tag="xrows")
                nc.gpsimd.indirect_dma_start(
                    out=x_rows[:], out_offset=None, in_=c0[:],
                    in_offset=bass.IndirectOffsetOnAxis(ap=dec_sb[:, 0:1], axis=0))
                xcol = wp.tile([128, 2 * BL], fp32, tag="xcol")
                for c in range(2):
                    tp = psA.tile([128, BL], fp32, tag="tpA")
                    nc.tensor.transpose(tp[:], x_rows[:, c * 128:(c + 1) * 128], ident[0:BL, 0:BL])
                    nc.vector.tensor_copy(xcol[:, c * BL:(c + 1) * BL], tp[:])

                gi_a = psA.tile([BL, 512], fp32, tag="gia")
                gi_b = psA.tile([BL, 256], fp32, tag="gib")
                gh_a = psA.tile([BL, 512], fp32, tag="gha")
                gh_b = psA.tile([BL, 256], fp32, tag="ghb")
                for c in range(2):
                    st, sp = (c == 0), (c == 1)
                    nc.tensor.matmul(gi_a[:], lhsT=xcol[:, c * BL:(c + 1) * BL],
                                     rhs=wiht_sb[:, c * 768:c * 768 + 512], start=st, stop=False)
                    nc.tensor.matmul(gi_b[:], lhsT=xcol[:, c * BL:(c + 1) * BL],
                                     rhs=wiht_sb[:, c * 768 + 512:c * 768 + 768], start=st, stop=False)
                    nc.tensor.matmul(gh_a[:], lhsT=hT_sb[:, c * BL:(c + 1) * BL],
                                     rhs=whht_sb[:, c * 768:c * 768 + 512], start=st, stop=False)
                    nc.tensor.matmul(gh_b[:], lhsT=hT_sb[:, c * BL:(c + 1) * BL],
                                     rhs=whht_sb[:, c * 768 + 512:c * 768 + 768], start=st, stop=False)
                # biases via K=1 matmul with ones
                nc.tensor.matmul(gi_a[:], lhsT=ones14[:], rhs=bih_sb[:, 0:512], start=False, stop=True)
                nc.tensor.matmul(gi_b[:], lhsT=ones14[:], rhs=bih_sb[:, 512:768], start=False, stop=True)
                nc.tensor.matmul(gh_a[:], lhsT=ones14[:], rhs=bhh_sb[:, 0:512], start=False, stop=True)
                nc.tensor.matmul(gh_b[:], lhsT=ones14[:], rhs=bhh_sb[:, 512:768], start=False, stop=True)

                r_sb = wp.tile([BL, E], fp32, tag="r")
                z_sb = wp.tile([BL, E], fp32, tag="z")
                n_sb = wp.tile([BL, E], fp32, tag="n")
                tmp = wp.tile([BL, E], fp32, tag="gtmp")
                gh_sb = wp.tile([BL, 3 * E], fp32, tag="ghsb")
                h_new = cp.tile([BL, E], fp32, tag="hnew")
                nc.vector.tensor_copy(gh_sb[:, 0:512], gh_a[:])
                nc.vector.tensor_copy(gh_sb[:, 512:768], gh_b[:])
                nc.vector.tensor_add(tmp[:], gi_a[:, 0:E], gh_sb[:, 0:E])
                nc.scalar.activation(r_sb[:], tmp[:], AF.Sigmoid)
                nc.vector.tensor_add(tmp[:], gi_a[:, E:2 * E], gh_sb[:, E:2 * E])
                nc.scalar.activation(z_sb[:], tmp[:], AF.Sigmoid)
                nc.vector.tensor_mul(tmp[:], r_sb[:], gh_sb[:, 2 * E:3 * E])
                nc.vector.tensor_add(tmp[:], tmp[:], gi_b[:])
                nc.scalar.activation(n_sb[:], tmp[:], AF.Tanh)
                # h_new = n + z*(h - n)
                nc.vector.tensor_sub(tmp[:], h_sb[:], n_sb[:])
                nc.vector.tensor_mul(tmp[:], tmp[:], z_sb[:])
                nc.vector.tensor_add(h_new[:], n_sb[:], tmp[:])

            # =========== per-batch: tree attention + story hops ===========
            with (
                tc.tile_pool(name="psB", bufs=2, space="PSUM") as psB,
                tc.tile_pool(name="psT", bufs=1, space="PSUM") as psT,
            ):

                def transpose_to(dst_ap, src_ap, psq=128):
                    pp = psT.tile([src_ap.shape[-1], src_ap.shape[0]], fp32, tag="tp")
                    nc.tensor.transpose(pp[:], src_ap,
                                        ident[0:src_ap.shape[0], 0:src_ap.shape[0]])
                    nc.vector.tensor_copy(dst_ap, pp[:])

                for b in range(BL):
                    # ---- tree/KB attention ----
                    kb_sb = wp.tile([128, nzt * L], i32, tag="kbo")
                    nc.sync.dma_start(kb_sb[:], kbv_off[b])
                    kt_sb = wp.tile([128, nzt], i32, tag="kto")
                    nc.sync.dma_start(kt_sb[:], ktype_off[b])
                    wm_sb = wp.tile([128, nzt * NT], fp32, tag="wm")
                    nc.sync.dma_start(wm_sb[:], wmat[b])

                    gkb = []
                    for tl in range(nzt):
                        g = wp.tile([128, E], fp32, tag=f"gkb{tl}")
                        for l in range(L):
                            nc.gpsimd.indirect_dma_start(
                                out=g[:], out_offset=None, in_=c0[:],
                                in_offset=bass.IndirectOffsetOnAxis(
                                    ap=kb_sb[:, tl * L + l:tl * L + l + 1], axis=0),
                                compute_op=(OP.bypass if l == 0 else OP.add))
                        trow = wp.tile([128, E], fp32, tag=f"trow{tl}")
                        nc.gpsimd.indirect_dma_start(
                            out=trow[:], out_offset=None, in_=t_tab[:],
                            in_offset=bass.IndirectOffsetOnAxis(
                                ap=kt_sb[:, tl:tl + 1], axis=0))
                        nc.vector.tensor_mul(g[:], g[:], trow[:])
                        gkb.append(g)

                    # rootsT [e, t] in 2 e-chunks
                    rootsT = wp.tile([128, 2 * NT], fp32, tag="rootsT")
                    for ec in range(2):
                        rp = psB.tile([128, NT], fp32, tag="attn")
                        for tl in range(nzt):
                            nc.tensor.matmul(rp[:], lhsT=gkb[tl][:, ec * 128:(ec + 1) * 128],
                                             rhs=wm_sb[:, tl * NT:(tl + 1) * NT],
                                             start=(tl == 0), stop=(tl == nzt - 1))
                        nc.vector.tensor_copy(rootsT[:, ec * NT:(ec + 1) * NT], rp[:])

                    # attention bias: -1e9 where sum_e roots[t,:] == 0
                    bias_ps = psB.tile([NT, 1], fp32, tag="attn")
                    for ec in range(2):
                        nc.tensor.matmul(bias_ps[:], lhsT=rootsT[:, ec * NT:(ec + 1) * NT],
                                         rhs=ones128[:], start=(ec == 0), stop=(ec == 1))
                    biasneg = wp.tile([NT, 1], fp32, tag="biasneg")
                    nc.vector.tensor_scalar(out=biasneg[:], in0=bias_ps[:],
                                            scalar1=0.0, scalar2=-1e9,
                                            op0=OP.is_equal, op1=OP.mult)

                    # keyrT [e', t] = Wk @ rootsT ; valr [t, e'] = rootsT^T @ WvT
                    keyrT = wp.tile([128, 2 * NT], fp32, tag="keyrT")
                    for ecp in range(2):
                        kp = psB.tile([128, NT], fp32, tag="attn")
                        for ec in range(2):
                            nc.tensor.matmul(
                                kp[:], lhsT=wkt_sb[:, ec * E + ecp * 128:ec * E + (ecp + 1) * 128],
                                rhs=rootsT[:, ec * NT:(ec + 1) * NT],
                                start=(ec == 0), stop=(ec == 1))
                        nc.vector.tensor_copy(keyrT[:, ecp * NT:(ecp + 1) * NT], kp[:])
                    valr = wp.tile([NT, E], fp32, tag="valr")
                    vp_ps = psB.tile([NT, E], fp32, tag="attn")
                    for ec in range(2):
                        nc.tensor.matmul(vp_ps[:], lhsT=rootsT[:, ec * NT:(ec + 1) * NT],
                                         rhs=wvt_sb[:, ec * E:(ec + 1) * E],
                                         start=(ec == 0), stop=(ec == 1))
                    nc.vector.tensor_copy(valr[:], vp_ps[:])

                    # query (column layout) for this batch elem
                    qcol = wp.tile([128, 2], fp32, tag="qcol")
                    for ecp in range(2):
                        qp = psB.tile([128, 1], fp32, tag="attn")
                        for ec in range(2):
                            nc.tensor.matmul(
                                qp[:], lhsT=wqt_sb[:, ec * E + ecp * 128:ec * E + (ecp + 1) * 128],
                                rhs=hT_sb[:, ec * BL + b:ec * BL + b + 1],
                                start=(ec == 0), stop=(ec == 1))
                        nc.vector.tensor_copy(qcol[:, ecp:ecp + 1], qp[:])

                    lg_ps = psB.tile([NT, 1], fp32, tag="attn")
                    for ecp in range(2):
                        nc.tensor.matmul(lg_ps[:], lhsT=keyrT[:, ecp * NT:(ecp + 1) * NT],
                                         rhs=qcol[:, ecp:ecp + 1],
                                         start=(ecp == 0), stop=(ecp == 1))
                    exps = wp.tile([NT, 1], fp32, tag="exps")
                    nc.scalar.activation(exps[:], lg_ps[:], AF.Exp, bias=biasneg[:])
                    z_ps = psB.tile([1, 1], fp32, tag="attn")
                    nc.tensor.matmul(z_ps[:], lhsT=exps[:], rhs=ones128[0:NT, :],
                                     start=True, stop=True)
                    zinv = wp.tile([1, 1], fp32, tag="zinv")
                    nc.vector.reciprocal(zinv[:], z_ps[:])
                    af_ps = psB.tile([1, E], fp32, tag="attn")
                    nc.tensor.matmul(af_ps[:], lhsT=exps[:], rhs=valr[:], start=True, stop=True)
                    af_row = wp.tile([1, E], fp32, tag="afrow")
                    nc.vector.tensor_scalar(out=af_row[:], in0=af_ps[:],
                                            scalar1=zinv[:], scalar2=None, op0=OP.mult)
                    for ec in range(2):
                        transpose_to(af_col[:, ec * BL + b:ec * BL + b + 1],
                                     af_row[:, ec * 128:(ec + 1) * 128])

                # cur_state (column layout) = h_new + attn_feat
                hnew_col = wp.tile([128, 2 * BL], fp32, tag="hnewcol")
                for ec in range(2):
                    transpose_to(hnew_col[:, ec * BL:(ec + 1) * BL],
                                 h_new[:, ec * 128:(ec + 1) * 128])
                nc.vector.tensor_add(cur_col[:], hnew_col[:], af_col[:])
                # back to rows for the cur_state output
                cur_rows = wp.tile([BL, E], fp32, tag="currows")
                for ec in range(2):
                    transpose_to(cur_rows[:, ec * 128:(ec + 1) * 128],
                                 cur_col[:, ec * BL:(ec + 1) * BL])
                nc.sync.dma_start(cur_o[:], cur_rows[:])

                # ---- story hops per batch elem ----
                for b in range(BL):
                    so_sb = wp.tile([128, MTILES * MT], i32, tag="so")
                    nc.sync.dma_start(so_sb[:], story_off[b])
                    vs = []
                    story_mode = os.environ.get('KERNEL_STORY', 'plain')
                    for mt in range(MTILES):
                        vtile = vp.tile([128, 4 * E], fp32, tag="vst")
                        if story_mode == 'cce':
                            for t in range(MT):
                                nc.gpsimd.indirect_dma_start(
                                    out=vtile[:], out_offset=None, in_=c_cat[:],
                                    in_offset=bass.IndirectOffsetOnAxis(
                                        ap=so_sb[:, mt * MT + t:mt * MT + t + 1], axis=0),
                                    compute_op=(OP.bypass if t == 0 else OP.add))
                            vs.append(vtile)
                            continue
                        # 4 independent gathers (full rate; CCE-accumulate halves
                        # the SBUF write bandwidth) + token-sum on DVE or PE
                        graw = []
                        for t in range(MT):
                            g = gp.tile([128, 4 * E], fp32, tag=f"graw{t}")
                            nc.gpsimd.indirect_dma_start(
                                out=g[:], out_offset=None, in_=c_cat[:],
                                in_offset=bass.IndirectOffsetOnAxis(
                                    ap=so_sb[:, mt * MT + t:mt * MT + t + 1], axis=0))
                            graw.append(g)
                        if mt % 4 != 3:
                            # token-sum on DVE
                            nc.vector.tensor_add(vtile[:], graw[0][:], graw[1][:])
                            nc.vector.tensor_add(vtile[:], vtile[:], graw[2][:])
                            nc.vector.tensor_add(vtile[:], vtile[:], graw[3][:])
                        else:
                            # token-sum on PE (identity matmul, accumulate in PSUM)
                            ts_ps = psT.tile([128, 4 * E], fp32, tag="tsum")
                            for c in range(2):
                                for t in range(MT):
                                    nc.tensor.matmul(
                                        ts_ps[:, c * 512:(c + 1) * 512],
                                        lhsT=ident[:],
                                        rhs=graw[t][:, c * 512:(c + 1) * 512],
                                        start=(t == 0), stop=(t == MT - 1))
                            nc.vector.tensor_copy(vtile[:], ts_ps[:])
                        vs.append(vtile)

                    # u maintained in row layout at partition 0; also feed ucat cols
                    u_row = wp.tile([1, E], fp32, tag="urow")
                    for ec in range(2):
                        nc.vector.tensor_copy(ucat[:, ec * BL + b:ec * BL + b + 1],
                                              cur_col[:, ec * BL + b:ec * BL + b + 1])
                        rp = psT.tile([1, 128], fp32, tag="tp")
                        nc.tensor.transpose(rp[:], cur_col[:, ec * BL + b:ec * BL + b + 1],
                                            ident[:, :])
                        nc.vector.tensor_copy(u_row[:, ec * 128:(ec + 1) * 128], rp[:])

                    for hop in range(HOPS):
                        # broadcast u across partitions: ubc[p, e] = u[e]
                        ub_ps = psT.tile([128, E], fp32, tag="ubc")
                        nc.tensor.matmul(ub_ps[:], lhsT=ones_row[:], rhs=u_row[:],
                                         start=True, stop=True)
                        ubc = wp.tile([128, E], fp32, tag="ubc_sb")
                        nc.vector.tensor_copy(ubc[:], ub_ps[:])

                        # keys on DVE: s[m] = sum_e m_story[hop][m,e] * u[e]
                        s_sb = wp.tile([128, MTILES], fp32, tag="ssb")
                        # NOTE: fused tensor_tensor_reduce hangs on HW here;
                        # the two-instruction form is reliable.
                        keys_mode = os.environ.get('KERNEL_KEYS', 'mulred')
                        for mt in range(MTILES):
                            scr = wp.tile([128, E], fp32, tag="scr")
                            if keys_mode == 'ttr':
                                nc.vector.tensor_tensor_reduce(
                                    out=scr[:], in0=vs[mt][:, hop * E:(hop + 1) * E],
                                    in1=ubc[:], scale=1.0, scalar=0.0,
                                    op0=OP.mult, op1=OP.add,
                                    accum_out=s_sb[:, mt:mt + 1])
                            else:
                                nc.vector.tensor_mul(scr[:], vs[mt][:, hop * E:(hop + 1) * E],
                                                     ubc[:])
                                nc.vector.tensor_reduce(
                                    out=s_sb[:, mt:mt + 1], in_=scr[:],
                                    axis=mybir.AxisListType.X, op=OP.add)

                        if hop == HOPS - 1:
                            # p_ptr = raw logits of the last hop
                            pp_ps = psT.tile([MTILES, 128], fp32, tag="tp")
                            nc.tensor.transpose(pp_ps[:], s_sb[:], ident[:, :])
                            pptr_sb = wp.tile([MTILES, 128], fp32, tag="pptr")
                            nc.vector.tensor_copy(pptr_sb[:], pp_ps[:])
                            nc.sync.dma_start(
                                p_ptr_o[:].rearrange("b (s c) -> b s c", s=MTILES)[b],
                                pptr_sb[:])
                            break

                        exp_sb = wp.tile([128, MTILES], fp32, tag="expsb")
                        rowsum = wp.tile([128, 1], fp32, tag="rowsum")
                        nc.scalar.activation(exp_sb[:], s_sb[:], AF.Exp, accum_out=rowsum[:])
                        z2_ps = psB.tile([1, 1], fp32, tag="attn")
                        nc.tensor.matmul(z2_ps[:], lhsT=rowsum[:], rhs=ones128[:],
                                         start=True, stop=True)
                        zinv2 = wp.tile([1, 1], fp32, tag="zinv2")
                        nc.vector.reciprocal(zinv2[:], z2_ps[:])

                        ok_ps = psB.tile([1, E], fp32, tag="ok")
                        for mt in range(MTILES):
                            nc.tensor.matmul(ok_ps[:], lhsT=exp_sb[:, mt:mt + 1],
                                             rhs=vs[mt][:, (hop + 1) * E:(hop + 2) * E],
                                             start=(mt == 0), stop=(mt == MTILES - 1))
                        okn = wp.tile([1, E], fp32, tag="okn")
                        nc.vector.tensor_scalar(out=okn[:], in0=ok_ps[:],
                                                scalar1=zinv2[:], scalar2=None, op0=OP.mult)
                        if hop == 0:
                            for ec in range(2):
                                transpose_to(ucat[:, (2 + ec) * BL + b:(2 + ec) * BL + b + 1],
                                             okn[:, ec * 128:(ec + 1) * 128])
                        # u <- u + o_k  (row space)
                        u_next = wp.tile([1, E], fp32, tag="urow")
                        nc.vector.tensor_add(u_next[:], u_row[:], okn[:])
                        u_row = u_next

            # ====== p_vocab: all-gather u_cat across cores, local vocab shard ======
            if os.environ.get('KERNEL_SKIP_W1'):
                zt = wp.tile([B, 512], fp32, tag="pvs")
                nc.vector.memset(zt[:], 0.0)
                for n0 in range(0, VS, 512):
                    nc.sync.dma_start(p_vocab_o[:, n0:n0 + min(512, VS - n0)],
                                      zt[:, 0:min(512, VS - n0)])
            else:
              with (
                tc.tile_pool(name="psC", bufs=2, space="PSUM") as psC,
                tc.tile_pool(name="dram", bufs=1, space="DRAM") as dram,
              ):
                bounce_in = dram.tile([128, 4 * BL], fp32)
                bounce_out = dram.tile([NCORES * 128, 4 * BL], fp32, addr_space="Shared")
                nc.gpsimd.dma_start(bounce_in[:], ucat[:])
                if os.environ.get('KERNEL_NO_COLL'):
                    # bisect mode: skip the collective (WRONG p_vocab for cores>0)
                    nc.gpsimd.dma_start(bounce_out[0:128, :], bounce_in[:])
                else:
                    nc.gpsimd.collective_compute(
                        "AllGather", OP.bypass,
                        ins=[bounce_in.opt()],
                        outs=[bounce_out.opt()],
                        replica_groups=[list(range(NCORES))],
                    )
                # ucat_w[p, jc*B + c*BL + b] = u_cat chunk jc of global batch (c, b)
                ucat_w = cp.tile([128, 4 * B], fp32, tag="ucat_w")
                for c in range(NCORES):
                    nc.sync.dma_start(
                        ucat_w[:].rearrange("p (j z) -> p j z", z=B)[:, :, c * BL:(c + 1) * BL],
                        bounce_out[c * 128:(c + 1) * 128, :].rearrange(
                            "p (j z) -> p j z", z=BL))

                NCH = (VS + 511) // 512
                for nch in range(NCH):
                    n0 = nch * 512
                    nsz = min(512, VS - n0)
                    pv = psC.tile([B, nsz], fp32, tag="pv")
                    for jc in range(4):
                        wtile = w1p.tile([128, nsz], fp32, tag="w1")
                        nc.sync.dma_start(wtile[:], w1t[jc * 128:(jc + 1) * 128, n0:n0 + nsz])
                        nc.tensor.matmul(pv[:], lhsT=ucat_w[:, jc * B:(jc + 1) * B],
                                         rhs=wtile[:], start=(jc == 0), stop=False)
                    btile = w1p.tile([1, nsz], fp32, tag="w1bt")
                    nc.sync.dma_start(btile[:], w1b[:, n0:n0 + nsz])
                    nc.tensor.matmul(pv[:], lhsT=ones132[:], rhs=btile[:],
                                     start=False, stop=True)
                    pvs = wp.tile([B, nsz], fp32, tag="pvs")
                    nc.vector.tensor_copy(pvs[:], pv[:])
                    nc.sync.dma_start(p_vocab_o[:, n0:n0 + nsz], pvs[:])

    nc.compile()
    return nc


# ----------------------------------------------------------------------------
# entry point
# ----------------------------------------------------------------------------
def kernel(**inputs):
    from concourse import bass_utils

    p = _host_prep(inputs)
    nzcap = p['nzcap']
    if nzcap not in _CACHE:
        _CACHE[nzcap] = _build(nzcap)
    nc = _CACHE[nzcap]

    VS = V // NCORES
    shared = {k: p[k] for k in ('c_cat', 'c0', 't_tab', 'wqt', 'wkt',
                                'wvt', 'wiht', 'whht', 'bih', 'bhh')}
    in_maps = []
    for c in range(NCORES):
        sl = slice(c * BL, (c + 1) * BL)
        m = dict(shared)
        m['w1t'] = np.ascontiguousarray(p['w1t'][:, c * VS:(c + 1) * VS])
        m['w1b'] = np.ascontiguousarray(p['w1b'][:, c * VS:(c + 1) * VS])
        m['story_off'] = p['story_off'][sl]
        m['kbv_off'] = p['kbv_off'][sl]
        m['ktype_off'] = p['ktype_off'][sl]
        m['wmat'] = p['wmat'][sl]
        m['dec_off'] = p['dec_off'][sl]
        m['h_rows'] = p['h_rows'][sl]
        m['hT'] = np.ascontiguousarray(p['hT'][:, :, sl].reshape(128, 2 * BL))
        in_maps.append(m)

    res = bass_utils.run_bass_kernel_spmd(nc, in_maps, core_ids=list(range(NCORES)))
    if res.exec_time_ns is not None:
        kernel.last_exec_time_ns = res.exec_time_ns

    p_ptr = np.concatenate([res.results[c]["p_ptr_o"] for c in range(NCORES)], 0)
    p_vocab = np.concatenate([res.results[c]["p_vocab_o"] for c in range(NCORES)], 1)
    cur = np.concatenate([res.results[c]["cur_o"] for c in range(NCORES)], 0)
    return p_ptr, p_vocab, cur[None]


kernel.last_exec_time_ns = None
